# revision 2
# baseline (speedup 1.0000x reference)
"""CKConv (nn_CKConv_85950885527678) Trainium2 Bass kernel.

Strategy: data-parallel over batch (8 batches -> 8 NeuronCores). The tiny
SIREN kernel network is evaluated on the host; the generated conv kernel is
replicated to every core (per the sharding hint).

Per core the causal conv out[o,t] = sum_{i,l>=1} K[o,i,l] * xp[i,t+l]
(xp = x left-padded with T zeros) is computed as a sequence of full-width
128x128 matmuls using a *diagonal* decomposition: because the causal
boundary t+l >= 2048 is a diagonal in (time, tap) space, four time-tiles
spaced S=64 apart can share one moving-operand slice, with each tile
receiving a different 4-tap block (shifted by S taps per tile). Each matmul
therefore has a full 128-column stationary operand (4 tiles x 32 out
channels), K=128 contraction (4 taps x 32 in channels), and N=64 moving
columns:

  pass p (times [256p, 256p+256)): psum[32P+o, c] accumulates over
  j = 0..64(p+1): exp[:, j, :].T @ XP[:, 256p+703-4j : +64]
  where exp[dd*32+i, j, 32P+o] = K[o, i, 4*(511-j+16P)+1+dd] (0 if the
  block index exceeds 511), yielding out[o, 256p + 64*(3-P) + c].

The stationary table is DMA'd in compact form (each tap block once,
reversed block order) and expanded 4x on-chip by the otherwise-idle DVE
engine, overlapped with PE compute. Bias adds run on the Activation engine.
"""

import os
import numpy as np

B, C_IN, C_OUT, T, D = 8, 32, 32, 2048, 32
L = T + 1
U0 = 1534
XPW = 2565
S = 64                  # time-tile size
NPASS = T // (4 * S)    # 8
NJ = 512                # stationary col-blocks
NBR = 608               # compact table blocks incl. 96 leading zero blocks
N_CORES = 8

_cache = {}


# ---------------------------------------------------------------- host prep

def _siren_kernel(pos_rel, w1, b1, w2, b2, w3, b3):
    p = pos_rel.reshape(1, L).astype(np.float32)
    h = np.sin(w1.astype(np.float32) @ p + b1[:, None].astype(np.float32))
    h = np.sin(w2.astype(np.float32) @ h + b2[:, None].astype(np.float32))
    k = w3.astype(np.float32) @ h + b3[:, None].astype(np.float32)
    return k.astype(np.float32)


def _build_wcr(k):
    """Compact reversed table (real blocks only): col (br-96)*32 + o holds
    K[o, i, 4b+1+dd] at partition dd*32+i, with b = 607-br, br in [96,608)."""
    kk = k.reshape(C_OUT, C_IN, L)[:, :, 1:]
    arr = kk.reshape(C_OUT, C_IN, NJ, 4).transpose(3, 1, 2, 0)  # [dd,i,b,o]
    w = arr.reshape(128, NJ, C_OUT)[:, ::-1, :]                 # b descending
    return np.ascontiguousarray(w.reshape(128, NJ * C_OUT)).astype(np.float32)


def _build_xp(x):
    xpad = np.zeros((B, C_IN, 2 * T + 8), np.float32)
    xpad[:, :, T : 2 * T] = x
    XP = np.empty((B, 128, XPW), np.float32)
    for dd in range(4):
        XP[:, dd * 32 : (dd + 1) * 32, :] = xpad[:, :, U0 + dd : U0 + dd + XPW]
    return XP


# ------------------------------------------------------- tile drain patch

def _patch_tile_drain():
    """This walrus build rejects >2 sync waits on a CTRL (Drain) instruction;
    spread the TileContext exit waits over single-wait NOPs instead."""
    from concourse.tile import TileContext
    from concourse.vector_clock import ScopedClock, VectorClock

    if getattr(TileContext, "_ck_drain_patched", False):
        return

    def _drain_and_barrier(self, tick_clock, wait_clock):
        gc = tick_clock.global_clock
        n = len(gc)
        for p in range(n):
            if gc[p] <= 0:
                continue
            vec = [gc[q] if q == p else 0 for q in range(n)]
            nop = self.nc.sync.nop(nofuse=True, hint=f"split_drain_wait_p{p}")
            wait_clock.add_sem_waits(nop.ins, ScopedClock({None: VectorClock(vec)}))
        self.nc.sync.drain()
        self.nc.all_engine_barrier()
        assert self.sems is not None
        popped = self.nc._tile_sem_poison_stack.pop()
        assert popped is self._sem_poison
        self.nc.clear_and_free_semaphores(list(self.sems.allocated().values()))
        self.nc.all_engine_barrier()

    TileContext._drain_and_barrier = _drain_and_barrier
    TileContext._ck_drain_patched = True


WAIT_LIMIT = 1  # this walrus build encodes at most 2 sync waits per instruction


def _split_excess_waits(nc, limit=WAIT_LIMIT):
    """Hoist excess sem waits onto same-engine NOPs placed just before the
    instruction — in-order engine queues make this semantically identical."""
    import concourse.mybir as mybir

    n_split = 0
    for f in nc.m.functions:
        for bb in f.blocks:
            new_insts = []
            changed = False
            for inst in bb.instructions:
                si = inst.sync_info
                waits = list(si.on_wait) if si is not None and si.on_wait else []
                if len(waits) > limit:
                    extra, keep = waits[:-limit], waits[-limit:]
                    for i in range(0, len(extra), limit):
                        n_split += 1
                        new_insts.append(
                            mybir.InstNoOp(
                                name=f"I-ckwsplit-{n_split}",
                                engine=inst.engine,
                                ins=[],
                                outs=[],
                                sync_info=mybir.SyncInfo(
                                    on_wait=extra[i : i + limit], on_update=[]
                                ),
                            )
                        )
                    inst.sync_info = mybir.SyncInfo(
                        on_wait=keep, on_update=list(si.on_update) if si.on_update else []
                    )
                    changed = True
                new_insts.append(inst)
            if changed:
                bb.instructions = new_insts
    return n_split


# ------------------------------------------------------------ device kernel

def _build_nc():
    import concourse.bass as bass
    import concourse.mybir as mybir
    from concourse.tile import TileContext

    _patch_tile_drain()
    f32 = mybir.dt.float32
    bf16 = mybir.dt.bfloat16

    nc = bass.Bass()
    xp_d = nc.declare_dram_parameter("xp", [128, XPW], bf16, isOutput=False)
    wcr_d = nc.declare_dram_parameter("wcr", [128, NJ * 32], bf16, isOutput=False)
    bias_d = nc.declare_dram_parameter("bias", [128, 1], f32, isOutput=False)
    out_d = nc.declare_dram_parameter("out", [128, NPASS * S], f32, isOutput=True)

    max_passes = int(os.environ.get("CK_MAX_PASSES", str(NPASS)))

    with TileContext(nc) as tc:
        with (
            tc.tile_pool(name="const", bufs=1) as const,
            tc.tile_pool(name="work", bufs=1) as work,
            tc.tile_pool(name="acc_psum", bufs=2, space="PSUM") as acc_psum,
        ):
            xp_sb = const.tile([128, XPW], bf16)
            bias_sb = const.tile([128, 1], f32)
            wcr_sb = const.tile([128, NBR, 32], bf16)
            exp_sb = const.tile([128, NJ, 128], bf16)
            out_sb = work.tile([128, NPASS * S], f32)

            # DMA order = first-use order. Pass 0 only needs xp cols
            # [448, 768) and compact blocks br [96, 160); cols [0, 451)
            # of xp are never read.
            nc.sync.dma_start(xp_sb[:, 448:768], xp_d[:, 448:768])
            nc.sync.dma_start(bias_sb[:, :], bias_d[:, :])
            nc.sync.dma_start(
                wcr_sb[:, 96:160, :],
                wcr_d[:, 0 : 64 * 32].rearrange("p (a b) -> p a b", b=32),
            )
            nc.sync.dma_start(xp_sb[:, 768:XPW], xp_d[:, 768:XPW])
            for m in range(1, 8):
                nc.sync.dma_start(
                    wcr_sb[:, 96 + 64 * m : 160 + 64 * m, :],
                    wcr_d[:, 64 * 32 * m : 64 * 32 * (m + 1)].rearrange(
                        "p (a b) -> p a b", b=32
                    ),
                )

            # zero region: blocks b >= 512 (br < 96)
            nc.vector.memset(wcr_sb[:, 0:96, :], 0.0)

            # on-chip 4x expansion on DVE: exp[:, j, 32P:+32] =
            # wcr[:, 96 + j - 16P, :], in 16 steps of 32 j each
            # (first-use order).
            for kstep in range(16):
                j0 = 32 * kstep
                for P in range(4):
                    nc.vector.tensor_copy(
                        exp_sb[:, j0 : j0 + 32, 32 * P : 32 * P + 32],
                        wcr_sb[:, 96 - 16 * P + j0 : 128 - 16 * P + j0, :],
                    )

            for p in range(max_passes):
                nj = S * (p + 1)
                acc = acc_psum.tile([128, S], f32)
                for r in range(nj):
                    cp = 256 * p + 703 - 4 * r
                    nc.tensor.matmul(
                        acc[:, :],
                        exp_sb[:, r, :],
                        xp_sb[:, cp : cp + S],
                        start=(r == 0),
                        stop=(r == nj - 1),
                    )
                # out = psum + bias on the Activation engine (DVE is busy
                # expanding)
                nc.scalar.activation(
                    out_sb[:, S * p : S * (p + 1)],
                    acc[:, :],
                    mybir.ActivationFunctionType.Identity,
                    bias=bias_sb[:, 0:1],
                )
            nc.sync.dma_start(out_d[:, :], out_sb[:, :])
    _split_excess_waits(nc)
    return nc


# ------------------------------------------------------------------- entry

def kernel(**inputs):
    import ml_dtypes
    from concourse.bass_utils import run_bass_kernel_spmd

    x = np.asarray(inputs["x"], dtype=np.float32)
    k = _siren_kernel(
        np.asarray(inputs["pos_rel"]), np.asarray(inputs["w1"]),
        np.asarray(inputs["b1"]), np.asarray(inputs["w2"]),
        np.asarray(inputs["b2"]), np.asarray(inputs["w3"]),
        np.asarray(inputs["b3"]),
    )
    WCR = _build_wcr(k).astype(ml_dtypes.bfloat16)
    XP = _build_xp(x).astype(ml_dtypes.bfloat16)
    bias = np.ascontiguousarray(
        np.tile(np.asarray(inputs["bias"], dtype=np.float32), 4).reshape(128, 1)
    )

    if "nc" not in _cache:
        _cache["nc"] = _build_nc()
    nc = _cache["nc"]

    n_cores = int(os.environ.get("CK_CORES", str(N_CORES)))
    in_maps = [
        {"xp": XP[b % B], "wcr": WCR, "bias": bias} for b in range(n_cores)
    ]

    def _gather(raw):
        # raw [128, 512]: psum group P holds times 256p + 64*(3-P) + c
        o4 = raw.reshape(4, 32, NPASS, S)[::-1]        # [Q=3-P, o, p, c]
        return np.ascontiguousarray(
            o4.transpose(1, 2, 0, 3).reshape(C_OUT, T)
        )

    # The axon-tunneled device occasionally throws a transient
    # NRT_EXEC_UNIT_UNRECOVERABLE on 8-core launches; retry, then fall back
    # to two 4-core waves (same NEFF, batches split across waves).
    res = None
    for attempt in range(3):
        try:
            res = run_bass_kernel_spmd(nc, in_maps, core_ids=list(range(n_cores)))
            break
        except Exception:
            if attempt == 2:
                res = None
            else:
                continue
    if res is not None:
        out = np.stack(
            [_gather(res.results[b % n_cores]["out"]) for b in range(B)], axis=0
        )
        return out.astype(np.float32)

    half = n_cores // 2 if n_cores > 1 else 1
    outs = []
    for w0 in range(0, B, half):
        wave_maps = [
            {"xp": XP[(w0 + c) % B], "wcr": WCR, "bias": bias}
            for c in range(half)
        ]
        wres = run_bass_kernel_spmd(nc, wave_maps, core_ids=list(range(half)))
        outs.extend(_gather(wres.results[c]["out"]) for c in range(half))
    out = np.stack(outs[:B], axis=0)
    return out.astype(np.float32)
